# revision 1
# baseline (speedup 1.0000x reference)
import numpy as np
import jax
import jax.numpy as jnp

F = 0.65
F_ACTIVE = 0.99
F_REST = 0.01
TAU_ADAPT = 1000.0
TAU_MEM = 10.0
DT = 1.0
ALPHA = 0.6
BETA = 0.4
SLOPE = 1.0
EPS = 1e-7

BATCH = 64
SEQ = 1024
INPUT_DIM = 256
UNITS = 256
N_CORES = 8


def _run_direction(x_tbd, W, U, b, R):
    u4 = W.shape[1]
    u = u4 // 4
    dtype = x_tbd.dtype
    Bsz = x_tbd.shape[1]

    def step(carry, x_t):
        h, c, Vm, Vth = carry
        z = x_t @ W + h @ U + b
        Vm_new = Vm + (DT / TAU_MEM) * (-Vm + R * z)
        v_scaled = Vm_new / (Vth + EPS)
        spikes = jax.nn.sigmoid(SLOPE * (v_scaled - 1.0))
        Vm_new = jnp.where(spikes > 0, jnp.zeros_like(Vm_new), Vm_new)
        r = jnp.clip(jnp.mean(spikes, axis=1, keepdims=True), 0.0, 1.0)
        f_adapt = jnp.where(jnp.abs(R * z) >= Vth, F_ACTIVE, F_REST)
        r_e = ALPHA * (r - F) + BETA * (r - jnp.mean(f_adapt, axis=1, keepdims=True))
        Vth_new = Vth + (DT / TAU_ADAPT) * r_e
        z0, z1, z2, z3 = jnp.split(spikes, 4, axis=1)
        i_g = jax.nn.sigmoid(z0)
        f_g = jax.nn.sigmoid(z1)
        c_new = f_g * c + i_g * jnp.tanh(z2)
        o_g = jax.nn.sigmoid(z3)
        h_new = o_g * jnp.tanh(c_new)
        return (h_new, c_new, Vm_new, Vth_new), h_new

    init = (
        jnp.zeros((Bsz, u), dtype),
        jnp.zeros((Bsz, u), dtype),
        jnp.zeros((Bsz, u4), dtype),
        jnp.zeros((Bsz, u4), dtype),
    )
    _, hs = jax.lax.scan(step, init, x_tbd)
    return hs


def _bilstm(x, Wf, Uf, bf, Rf, Wb, Ub, bb, Rb):
    x_tbd = jnp.swapaxes(x, 0, 1)
    fwd = _run_direction(x_tbd, Wf, Uf, bf, Rf)
    bwd = _run_direction(x_tbd[::-1], Wb, Ub, bb, Rb)
    out = jnp.concatenate([fwd, bwd], axis=-1)
    return jnp.swapaxes(out, 0, 1)


def kernel(**inputs):
    x = np.asarray(inputs["x"], dtype=np.float32)
    ws = tuple(
        np.asarray(inputs[k], dtype=np.float32)
        for k in ("Wf", "Uf", "bf", "Rf", "Wb", "Ub", "bb", "Rb")
    )

    # Data-parallel over batch: shard x's batch axis (64 -> 8 per core)
    # across the 8 NeuronCores, replicate the small weight matrices.
    try:
        devs = jax.devices()
        if len(devs) >= N_CORES and devs[0].platform != "cpu":
            pm = jax.pmap(
                _bilstm,
                in_axes=(0,) + (None,) * 8,
                devices=devs[:N_CORES],
            )
            xs = x.reshape(N_CORES, BATCH // N_CORES, SEQ, INPUT_DIM)
            out = pm(xs, *ws)
            out = np.asarray(out, dtype=np.float32).reshape(BATCH, SEQ, 2 * UNITS)
            return out
    except Exception:
        pass

    # Fallback: single-device CPU execution.
    cpu = jax.devices("cpu")[0]
    args = [jax.device_put(a, cpu) for a in (x,) + ws]
    out = jax.jit(_bilstm, device=cpu)(*args)
    return np.asarray(out, dtype=np.float32)
